# revision 54
# baseline (speedup 1.0000x reference)
"""Trainium2 Bass kernel for nn_BasicResidualBlock (spiking CNN block).

Computation (per reference):
    s1 = IF_scan(x)                 # v += x; s = H(v-1); v *= (1-s)
    y1 = conv3x3(s1, w1) * inv1 + shift1
    s2 = IF_scan(y1)
    out = conv3x3(s2, w2) * inv2 + shift2

Shapes: x [T=8, B=32, C=128, H=32, W=32] fp32.

Strategy:
  - Data-parallel over B across 8 cores (4 images per core).
  - Per (t, b) image: channels C=128 on SBUF partitions, H*W on the free dim.
  - IF neuron state v kept in a zero-padded [128, 34*34] layout so the 3x3
    conv taps can read shifted windows directly (pad border stays exactly 0
    through the IF ops: is_ge(0,1)=0, (0 is_lt 1)*0 = 0).
  - conv3x3 = 9 shifted matmuls accumulating in PSUM; spikes are exactly
    representable in any matmul dtype, so products w*s are exact up to the
    weight rounding. BN scale is folded into the weights host-side; BN shift
    is a per-channel bias applied on the PSUM->SBUF copy / v2 accumulate.
  - conv1 weights: f32r hi (PE rounds fp32 operands to 12 mantissa bits —
    probed on HW) + an e4m3 lo-correction on 6 of 9 taps, run as 3 fp8
    DoubleRow tap-pairs per PSUM half (~0.57 cyc/tap-col vs 1.0 for bf16).
    The DR rhs pair stride comes from two shifted fp8 spike copies laid
    SX apart (copy B = copy A shifted one spike row), so pair j reads taps
    (d, d+34). Uncorrected-tap rounding costs ~1.6e-2 rel err via spike
    flips at IF2 (vs the 2e-2 gate); the 12->16 bit correction on 6 taps is
    the cheapest point that passes. conv2 feeds the output directly (no
    threshold amplification) and uses a single bf16 split.
  - Software-pipelined: conv2/output of image i-1 is emitted between conv1
    of image i and i+1 so the PE never waits on the vector-engine IF ops.
    PSUM banks: conv1-hi halves double-buffered (4) + lo halves (2) +
    conv2 halves (2). fp8 spike copies are written by the scalar engine
    (GpSimd converts fp32->fp8 via a ~20us software path - avoid).
"""

import sys

import numpy as np

try:
    import concourse  # noqa: F401
except ImportError:
    for _p in ("/opt/trn_rl_repo", "/root/.axon_site/_ro/trn_rl_repo"):
        if _p not in sys.path:
            sys.path.insert(0, _p)

import ml_dtypes

EPS = 1e-5
N_CORES = 8
T, B, C, H, W = 8, 32, 128, 32, 32
NB = B // N_CORES          # images per core per timestep
N_IMGS = T * NB            # images per core
HW = H * W                 # 1024
PW = W + 2                 # padded row width 34
PHW = (H + 2) * PW         # 1156
# conv1 weight modes (probed on HW: f32r = round-to-nearest at 12 mantissa
# bits incl implicit; the spike-threshold amplification of that rounding
# measures 2.13e-2 rel err — just over the 2e-2 gate — so f32r alone fails):
#   "bf16x2"   2-way bf16 hi/lo split, 36 bf16 matmuls/img   (~399us, 5.7e-3)
#   "f32r"     18 f32r matmuls/img                           (~288us, 2.1e-2)
#   "f32r+fp8" f32r hi + e4m3 lo-correction (scale 2^16, ~16-bit effective)
#              run as fp8 DoubleRow pairs: +10 matmuls/img   (sim 3.8e-3)
CONV1_MODE = "f32r+fp8"    # "f32r+fp8" | "f32r" | "bf16x2"
LO_SCALE = 2.0 ** 16       # e4m3 lo-split scale (host multiplies, PSUM-combine
                           # divides; power of two => exact)
SX = 1168                  # fp8 spike-copy stride, 16B-aligned (> PHW=1156)
# How many conv1 taps get the fp8 lo-correction. Spike-flip error grows as
# sqrt(#uncorrected taps): 0 corrected = 2.13e-2 (fails the 2e-2 gate),
# 9 corrected = 5.6e-3. 6 (rows 0-1) = 3 clean DoubleRow pairs per half,
# drops the third spike copy and the plain-fp8 matmul entirely.
LO_TAPS = 6                # 6 | 9
CONV2_SPLITS = 1
# conv2 weight dtype: "f32r" as the MOVING-side dtype (both operands f32r)
# measured ~11% slower than bf16 e2e (416us vs 402us) — keep bf16 rhs.
CONV2_MODE = "bf16"

_program_cache = {}


def build_program(n_imgs=N_IMGS, n_b=NB, conv1_mode=CONV1_MODE,
                  conv2_splits=CONV2_SPLITS, conv2_mode=CONV2_MODE):
    import concourse.mybir as mybir
    from concourse.bacc import Bacc
    from concourse.tile import TileContext

    f32 = mybir.dt.float32
    bf16 = mybir.dt.bfloat16
    f32r = mybir.dt.float32r
    Alu = mybir.AluOpType
    Act = mybir.ActivationFunctionType
    fp8 = mybir.dt.float8e4
    DR = mybir.MatmulPerfMode.DoubleRow
    use_f32r = conv1_mode.startswith("f32r")
    conv1_lo = conv1_mode == "f32r+fp8"
    conv1_splits = 1 if use_f32r else 2
    w1_dt = f32r if use_f32r else bf16
    # walrus rejects mixed 32/16-bit matmul inputs, so the conv1 spikes are
    # stored f32r whenever the weights are (the BIR verifier requires every
    # producer feeding an f32r matmul to round its output to f32r, so the
    # DVE spike writes must target an f32r-typed tile). Memset can't encode
    # f32r, so those go through an f32 bitcast view of the same memory.
    s1_dt = f32r if use_f32r else bf16
    s1_ms = ((lambda s: s.bitcast(f32)) if use_f32r
             else (lambda s: s))
    w2_dt = f32r if conv2_mode == "f32r" else bf16
    s2_dt = f32r if conv2_mode == "f32r" else bf16

    nc = Bacc()
    x_d = nc.declare_dram_parameter("x", [n_imgs, C, HW], f32, isOutput=False)
    w1_d = nc.declare_dram_parameter("w1", [C, 9 * conv1_splits * C], w1_dt,
                                     isOutput=False)
    if conv1_lo:
        w1l_d = nc.declare_dram_parameter("w1l", [C, LO_TAPS * C], fp8,
                                          isOutput=False)
    w2_d = nc.declare_dram_parameter("w2", [C, 9 * conv2_splits * C], w2_dt,
                                     isOutput=False)
    b1_d = nc.declare_dram_parameter("b1", [C, 1], f32, isOutput=False)
    b2_d = nc.declare_dram_parameter("b2", [C, 1], f32, isOutput=False)
    y_d = nc.declare_dram_parameter("y", [n_imgs, C, HW], f32, isOutput=True)

    with TileContext(nc) as tc:
        with (
            tc.tile_pool(name="const", bufs=1) as cp,
            tc.tile_pool(name="state", bufs=1) as vp,
            tc.tile_pool(name="work", bufs=2) as wp,
            tc.tile_pool(name="psum", bufs=4, space="PSUM") as pp,
        ):
            # First input image before the (larger) weight blobs so the
            # startup-critical path (x0 -> IF -> first matmul) isn't queued
            # behind them; memsets go to the otherwise-idle GpSimd engine.
            # x0 arrives in two chunks split at spike row 17 so the first
            # PSUM-half matmuls (needing rows 0..16) start before the full
            # image lands.
            # Each dma_start costs ~650ns of serial issue time on its queue,
            # so the startup-critical transfers are spread over two HWDGE
            # queues: x0 chunks issue on Sync while the w1 blocks issue in
            # parallel on the (otherwise idle until ~28us) Scalar queue.
            X0SPLIT = 17 * W
            x0 = wp.tile([C, HW], f32, tag="xt", bufs=3, name="xt_0")
            w1s = cp.tile([C, 9 * conv1_splits * C], w1_dt, tag="w1s", name="w1s")
            w2s = cp.tile([C, 9 * conv2_splits * C], w2_dt, tag="w2s", name="w2s")
            b1s = cp.tile([C, 1], f32, tag="b1s", name="b1s")
            b2s = cp.tile([C, 1], f32, tag="b2s", name="b2s")
            hc = 9 * C  # one split-block of weight columns
            nc.sync.dma_start(out=x0[:, :X0SPLIT], in_=x_d[0][:, :X0SPLIT])
            for s_ in range(conv1_splits):
                nc.scalar.dma_start(out=w1s[:, s_ * hc:(s_ + 1) * hc],
                                    in_=w1_d[:, s_ * hc:(s_ + 1) * hc])
            nc.sync.dma_start(out=x0[:, X0SPLIT:], in_=x_d[0][:, X0SPLIT:])
            nc.scalar.dma_start(out=b1s, in_=b1_d[:, :])
            if conv1_lo:
                w1ls = cp.tile([C, LO_TAPS * C], fp8, tag="w1ls", name="w1ls")
                nc.scalar.dma_start(out=w1ls, in_=w1l_d[:, :])
            # w2/b2 are first needed ~25us in (conv2 of image 0); their DMAs
            # are emitted after image 0 so they don't steal HBM bandwidth
            # from the startup-critical x0/w1 transfers.
            def load_conv2_weights():
                nc.scalar.dma_start(out=w2s, in_=w2_d[:, :])
                nc.scalar.dma_start(out=b2s, in_=b2_d[:, :])

            v1 = [vp.tile([C, PHW], f32, tag=f"v1_{b}", name=f"v1_{b}")
                  for b in range(n_b)]
            v2 = [vp.tile([C, PHW], f32, tag=f"v2_{b}", name=f"v2_{b}")
                  for b in range(n_b)]
            # t=0 spike tiles are written interior-only (see below), so zero
            # them fully up front; s1 of image 0 goes first on the GpSimd
            # queue because it gates the very first matmul.
            s1_first = [wp.tile([C, PHW], s1_dt, tag="s1", bufs=3,
                                name=f"s1_{i}") for i in range(n_b)]
            for s in s1_first:
                nc.gpsimd.memset(s1_ms(s), 0.0)
            for b in range(n_b):
                nc.gpsimd.memset(v1[b], 0.0)
            for b in range(n_b):
                nc.gpsimd.memset(v2[b], 0.0)

            # Warm the PE's HAM clock gate during the startup DMA window with
            # throwaway matmuls on a zeroed tile (cold PE runs at 1.2 GHz for
            # ~3.4us of activity; this burns that ramp on dead time). Scratch
            # PSUM reuses the ps2 slots, which sit idle until ~28us.
            # PSUM bank budget (8 banks of [128, 512]f32): hi conv1 halves
            # double-buffered (4) + lo halves (2) + conv2 halves (2). Without
            # the lo path conv2 keeps its double buffer.
            ps2_bufs = 2 if conv1_lo else 4
            wdum = cp.tile([C, HW // 2], bf16, tag="wdum", name="wdum")
            nc.vector.memset(wdum, 0.0)
            for k in range(14):
                psw = pp.tile([C, HW // 2], f32, tag="ps2", bufs=ps2_bufs,
                              name=f"warm_{k}")
                nc.tensor.matmul(out=psw, lhsT=wdum[:, 0:C], rhs=wdum,
                                 start=True, stop=True)

            def if_stage(v, src, s_tile):
                # v: padded state [C, PHW]; src: [C, HW]; s_tile: [C, PHW] bf16
                vv = v.rearrange("p (h w) -> p h w", w=PW)
                sv = src.rearrange("p (h w) -> p h w", w=W)
                nc.vector.tensor_tensor(
                    out=vv[:, 1:H + 1, 1:W + 1], in0=vv[:, 1:H + 1, 1:W + 1],
                    in1=sv, op=Alu.add)
                nc.vector.tensor_scalar(
                    out=s_tile, in0=v, scalar1=1.0, scalar2=None, op0=Alu.is_ge)
                nc.vector.scalar_tensor_tensor(
                    out=v, in0=v, scalar=1.0, in1=v, op0=Alu.is_lt, op1=Alu.mult)

            def conv(s_tile, w_sb, n_splits, psum_tag, halves_inner=True,
                     psum_bufs=4):
                # Weight layout is split-major ([split][tap] blocks of C cols)
                # so the hi-split matmuls — emitted first — only depend on the
                # first half of the weight blob. halves_inner pairs the two
                # PSUM-half matmuls per weight tap (consecutive same lhsT);
                # image 0's conv1 uses half-major order instead so its first
                # matmuls only need the first spike rows.
                sv = s_tile.rearrange("p (h w) -> p h w", w=PW)
                nmm = 9 * n_splits
                halves = [pp.tile([C, HW // 2], f32, tag=psum_tag,
                                  bufs=psum_bufs,
                                  name=f"{psum_tag}_{h2}") for h2 in range(2)]
                order = (
                    [(h2, s_, ky, kx)
                     for s_ in range(n_splits) for ky in range(3)
                     for kx in range(3) for h2 in range(2)]
                    if halves_inner else
                    [(h2, s_, ky, kx)
                     for h2 in range(2) for s_ in range(n_splits)
                     for ky in range(3) for kx in range(3)])
                for h2, s_, ky, kx in order:
                    idx = s_ * 9 + ky * 3 + kx
                    col = idx * C
                    r0 = (H // 2) * h2 + ky
                    nc.tensor.matmul(
                        out=halves[h2],
                        lhsT=w_sb[:, col:col + C],
                        rhs=sv[:, r0:r0 + H // 2, kx:kx + W],
                        start=(idx == 0), stop=(idx == nmm - 1))
                return halves

            n_copies = 3 if LO_TAPS == 9 else 2

            def make_s8(s1):
                # fp8 copies of the spike field at flat shifts 0/34(/35),
                # laid SX apart, so fp8 DoubleRow matmuls can pair two conv
                # taps via the copy stride: pair (j, j+1) at flat base d reads
                # taps (d, d+34) from (A,B) or (d+34, d+35) from (B,C).
                # GpSimd converts fp32->fp8 through a ~20us software path, so
                # the copies go to the scalar engine (native convert, idle
                # outside the PSUM evictions) with the third on the DVE.
                s8 = wp.tile([C, n_copies * SX], fp8, tag="s8", bufs=3,
                             name="s8")
                sf = s1.bitcast(f32)
                nc.scalar.activation(
                    out=s8[:, 0:PHW], in_=sf[:, 0:PHW],
                    func=Act.Identity, scale=1.0)
                nc.scalar.activation(
                    out=s8[:, SX:SX + PHW - 34], in_=sf[:, 34:PHW],
                    func=Act.Identity, scale=1.0)
                if n_copies == 3:
                    nc.vector.tensor_scalar(
                        out=s8[:, 2 * SX:2 * SX + PHW - 35], in0=sf[:, 35:PHW],
                        scalar1=0.0, scalar2=None, op0=Alu.add)
                return s8

            # lo-blob pair layout: [(t00,t10), (t01,t11), (t02,t12),
            # (t20,t21), t22] as 2C-wide column blocks (+ final C block).
            def conv_lo(s8):
                halves = [pp.tile([C, HW // 2], f32, tag="ps1l", bufs=2,
                                  name=f"ps1l_{h2}") for h2 in range(2)]
                sj = s8[:, 0:n_copies * SX].rearrange("p (j x) -> p j x",
                                                      j=n_copies)
                sjv = sj[:, :, 0:PHW].rearrange("p j (h w) -> p j h w", w=PW)
                av = s8[:, 0:PHW].rearrange("p (h w) -> p h w", w=PW)
                for h2 in range(2):
                    r0 = (H // 2) * h2
                    for k in range(3):   # pairs (t0k, t1k) from copies (A,B)
                        lw = w1ls[:, 2 * k * C:(2 * k + 2) * C].rearrange(
                            "p (j c) -> p j c", j=2)
                        nc.tensor.matmul(
                            out=halves[h2], lhsT=lw,
                            rhs=sjv[:, 0:2, r0:r0 + H // 2, k:k + W],
                            start=(k == 0), stop=(LO_TAPS == 6 and k == 2),
                            perf_mode=DR)
                    if LO_TAPS == 9:
                        # pair (t20, t21) from copies (B,C): base row r0+1
                        lw = w1ls[:, 6 * C:8 * C].rearrange(
                            "p (j c) -> p j c", j=2)
                        nc.tensor.matmul(
                            out=halves[h2], lhsT=lw,
                            rhs=sjv[:, 1:3, r0 + 1:r0 + 1 + H // 2, 0:W],
                            start=False, stop=False, perf_mode=DR)
                        # single t22 as a plain fp8 matmul from copy A
                        nc.tensor.matmul(
                            out=halves[h2], lhsT=w1ls[:, 8 * C:9 * C],
                            rhs=av[:, r0 + 2:r0 + 2 + H // 2, 2:2 + W],
                            start=False, stop=True)
                return halves

            pending = {}
            pending_lo = {}
            for i in range(n_imgs + 1):
                if i < n_imgs:
                    b = i % n_b
                    if i == 0:
                        xt = x0
                    else:
                        xt = wp.tile([C, HW], f32, tag="xt", bufs=3,
                                     name=f"xt_{i}")
                        nc.sync.dma_start(out=xt, in_=x_d[i])
                    if i < n_b:
                        # t == 0: v is zero, so spike/reset come straight from
                        # x (skips the accumulate on the startup-critical path;
                        # s border is zeroed by an early gpsimd memset instead
                        # of inherited from the padded v state).
                        s1 = s1_first[i]
                        vv = v1[b].rearrange("p (h w) -> p h w", w=PW)
                        xv = xt.rearrange("p (h w) -> p h w", w=W)
                        s1v = s1.rearrange("p (h w) -> p h w", w=PW)
                        if i == 0:
                            # Split at row 17 to match the x0 DMA chunks:
                            # spikes for PSUM half 0 don't wait on chunk B.
                            nc.vector.tensor_scalar(
                                out=s1v[:, 1:18, 1:W + 1], in0=xv[:, 0:17, :],
                                scalar1=1.0, scalar2=None, op0=Alu.is_ge)
                            nc.vector.tensor_scalar(
                                out=s1v[:, 18:H + 1, 1:W + 1],
                                in0=xv[:, 17:H, :],
                                scalar1=1.0, scalar2=None, op0=Alu.is_ge)
                        else:
                            nc.vector.tensor_scalar(
                                out=s1v[:, 1:H + 1, 1:W + 1], in0=xv,
                                scalar1=1.0, scalar2=None, op0=Alu.is_ge)
                        nc.vector.scalar_tensor_tensor(
                            out=vv[:, 1:H + 1, 1:W + 1], in0=xv, scalar=1.0,
                            in1=xv, op0=Alu.is_lt, op1=Alu.mult)
                    else:
                        s1 = wp.tile([C, PHW], s1_dt, tag="s1", bufs=3,
                                     name=f"s1_{i}")
                        if_stage(v1[b], xt, s1)
                    if conv1_lo:
                        s8_cur = make_s8(s1)
                    pending[i] = conv(s1, w1s, conv1_splits, "ps1",
                                      halves_inner=(i != 0))
                    if i == 0:
                        if conv1_lo:
                            pending_lo[0] = conv_lo(s8_cur)
                        load_conv2_weights()
                if i >= 1:
                    j = i - 1
                    b = j % n_b
                    ps1 = pending.pop(j)
                    # v2 += conv1_out + shift1, straight from PSUM (one DVE op
                    # per half; no intermediate SBUF copy needed)
                    v2v = v2[b].rearrange("p (h w) -> p h w", w=PW)
                    ps1l = pending_lo.pop(j, None)
                    for h2 in range(2):
                        vint = v2v[:, 1 + (H // 2) * h2:1 + (H // 2) * (h2 + 1),
                                   1:W + 1]
                        nc.vector.scalar_tensor_tensor(
                            out=vint, in0=ps1[h2].rearrange(
                                "p (h w) -> p h w", w=W),
                            scalar=b1s[:, 0:1], in1=vint,
                            op0=Alu.add, op1=Alu.add)
                        if ps1l is not None:
                            nc.vector.scalar_tensor_tensor(
                                out=vint, in0=ps1l[h2].rearrange(
                                    "p (h w) -> p h w", w=W),
                                scalar=1.0 / LO_SCALE, in1=vint,
                                op0=Alu.mult, op1=Alu.add)
                    s2 = wp.tile([C, PHW], s2_dt, tag="s2", bufs=3,
                                 name=f"s2_{j}")
                    nc.vector.tensor_scalar(
                        out=s2, in0=v2[b], scalar1=1.0, scalar2=None,
                        op0=Alu.is_ge)
                    nc.vector.scalar_tensor_tensor(
                        out=v2[b], in0=v2[b], scalar=1.0, in1=v2[b],
                        op0=Alu.is_lt, op1=Alu.mult)
                    ps2 = conv(s2, w2s, conv2_splits, "ps2",
                               psum_bufs=ps2_bufs)
                    ot = wp.tile([C, HW], f32, tag="ot", bufs=3, name=f"ot_{j}")
                    # NOTE: quarter-granularity eviction was measured 33us
                    # SLOWER: the extra scalar-engine ops queue between the
                    # s8 fp8-converts and delay the DoubleRow matmuls.
                    for h2 in range(2):
                        sl = slice(h2 * (HW // 2), (h2 + 1) * (HW // 2))
                        nc.scalar.activation(
                            out=ot[:, sl], in_=ps2[h2], func=Act.Identity,
                            bias=b2s[:, 0:1], scale=1.0)
                        nc.sync.dma_start(out=y_d[j][:, sl], in_=ot[:, sl])
                    # Image i's lo matmuls are emitted after conv2(i-1), so
                    # the single-buffered lo PSUM slots and the scalar-engine
                    # fp8 spike copies get a full conv2 of extra slack.
                    if conv1_lo and 1 <= i < n_imgs:
                        pending_lo[i] = conv_lo(s8_cur)

    nc.finalize()
    return nc


def _f32r_round(w):
    # Probed on HW: the PE reads f32r operands rounded to nearest at 12
    # mantissa bits (incl implicit). Applying the same rounding host-side
    # makes the stored hi split exactly what the PE multiplies by, so the
    # fp8 lo split corrects the true residual.
    m, e = np.frexp(np.asarray(w, np.float64))
    return np.ldexp(np.round(m * (1 << 12)) / (1 << 12), e)


def _split_weights(wf, n_splits, dtype):
    # wf: [O, I, 3, 3] float64 (BN scale already folded)
    lhsT = np.transpose(wf, (2, 3, 1, 0)).reshape(9, C, C)  # [tap, ci, co]
    comps, rem = [], lhsT.copy()
    for _ in range(n_splits):
        c = (_f32r_round(rem).astype(np.float32) if dtype == np.float32
             else rem.astype(dtype))
        comps.append(np.asarray(c))
        rem = rem - c.astype(np.float64)
    # [split, tap, ci, co] -> [ci, split, tap, co] -> [ci, split*tap*co]
    a = np.stack(comps, axis=0)
    return np.ascontiguousarray(
        a.transpose(2, 0, 1, 3).reshape(C, 9 * n_splits * C))


# conv_lo pairs taps ((0,k),(1,k)) for k=0..2 [, then ((2,0),(2,1)), (2,2)]
_LO_TAP_ORDER = [0, 3, 1, 4, 2, 5, 6, 7, 8][:LO_TAPS]


def _lo_split(wf, hi_blob_taps):
    # e4m3 residual correction, scaled by LO_SCALE (host) / un-scaled in the
    # PSUM combine. Column blocks follow _LO_TAP_ORDER.
    lhsT = np.transpose(wf, (2, 3, 1, 0)).reshape(9, C, C)
    rem = lhsT - hi_blob_taps
    lo = np.asarray((rem * LO_SCALE).astype(ml_dtypes.float8_e4m3fn))
    lo = lo[_LO_TAP_ORDER]
    return np.ascontiguousarray(lo.transpose(1, 0, 2).reshape(C, LO_TAPS * C))


def _prep(w, g, b, m, v, n_splits, dtype=ml_dtypes.bfloat16, want_lo=False):
    inv = g.astype(np.float64) / np.sqrt(v.astype(np.float64) + EPS)
    wf = w.astype(np.float64) * inv[:, None, None, None]
    shift = (b.astype(np.float64) - m.astype(np.float64) * inv)
    blob = _split_weights(wf, n_splits, dtype)
    out = (blob, shift.astype(np.float32).reshape(C, 1))
    if want_lo:
        hi_taps = _f32r_round(
            np.transpose(wf, (2, 3, 1, 0)).reshape(9, C, C))
        out = out + (_lo_split(wf, hi_taps),)
    return out


last_results = None  # BassKernelResults of the most recent run (for test.py)

# Note: walrus --enable-ldw-opt=true was tried to elide the redundant weight
# load of each same-lhsT matmul pair; the compiler rejects this kernel's
# Ldweights form ("InstLdweights is not compatible with LDW optimization"),
# so the ~6ns/matmul weight-load issue tax is a hard floor here.


def kernel(x, w1, g1, b1, m1, v1, w2, g2, b2, m2, v2, _trace=False):
    global last_results
    from concourse.bass_utils import run_bass_kernel_spmd

    x = np.asarray(x)
    assert x.shape == (T, B, C, H, W), x.shape

    key = (CONV1_MODE, CONV2_SPLITS, CONV2_MODE)
    if key not in _program_cache:
        _program_cache[key] = build_program(
            conv1_mode=CONV1_MODE, conv2_splits=CONV2_SPLITS,
            conv2_mode=CONV2_MODE)
    nc = _program_cache[key]

    use_f32r = CONV1_MODE.startswith("f32r")
    want_lo = CONV1_MODE == "f32r+fp8"
    conv1_splits = 1 if use_f32r else 2
    w1_np = np.float32 if use_f32r else ml_dtypes.bfloat16
    w2_np = np.float32 if CONV2_MODE == "f32r" else ml_dtypes.bfloat16
    p1 = _prep(np.asarray(w1), np.asarray(g1), np.asarray(b1),
               np.asarray(m1), np.asarray(v1), conv1_splits,
               dtype=w1_np, want_lo=want_lo)
    w1p, sh1 = p1[0], p1[1]
    w2p, sh2 = _prep(np.asarray(w2), np.asarray(g2), np.asarray(b2),
                     np.asarray(m2), np.asarray(v2), CONV2_SPLITS,
                     dtype=w2_np)

    in_maps = []
    for c in range(N_CORES):
        xs = np.ascontiguousarray(
            x[:, c * NB:(c + 1) * NB].reshape(N_IMGS, C, HW))
        m_ = {"x": xs, "w1": w1p, "w2": w2p, "b1": sh1, "b2": sh2}
        if want_lo:
            m_["w1l"] = p1[2]
        in_maps.append(m_)

    last_results = run_bass_kernel_spmd(
        nc, in_maps, list(range(N_CORES)), trace=_trace)
    res = last_results.results
    out = np.empty((T, B, C, H, W), np.float32)
    for c in range(N_CORES):
        out[:, c * NB:(c + 1) * NB] = res[c]["y"].reshape(T, NB, C, H, W)
    return out

